# revision 70
# baseline (speedup 1.0000x reference)
"""Multi-head causal attention (B=4, S=2048, H=1024, NH=16) on 8 trn2 cores.

Sharding: core = (batch b, head-group g) with 4 batches x 2 groups; each core
computes 8 heads of one batch.  Host sums the 2 group partials per batch and
adds the output bias.

Per-core pipeline (dtype plan driven by fp8-e4m3 error measurements):
 - Q/K projections in fp8 DoubleRow (x split hi+lo planes on host for error
   compensation; weights single-quantized, duplicated across planes).
 - V^T computed directly via orientation swap (x^T chunks stationary, Wv
   moving) in bf16 -- no on-device V transpose.
 - Scores S^T[k,q] per head via fp8 DoubleRow with K compensated hi/lo planes
   and Q broadcast across planes.
 - exp on ACT -> P bf16 tiles; causal mask multiplied on the diagonal chunks.
 - P@V with P as the *stationary* operand [k,q] and V^T [k,64+ones] moving:
   full PE rate in bf16, rowsum for free; per-partition normalize.
 - ctx^T -> ctx via PE transpose; output projection bf16; bf16 partial out.
"""
import numpy as np
import ml_dtypes

import concourse.bacc as bacc
import concourse.tile as tile
from concourse import mybir
from concourse.bass_utils import run_bass_kernel_spmd

F32 = mybir.dt.float32
BF16 = mybir.dt.bfloat16
FP8 = mybir.dt.float8e4
AF = mybir.ActivationFunctionType
DR = mybir.MatmulPerfMode.DoubleRow
MUL = mybir.AluOpType.mult
ADD = mybir.AluOpType.add
SUB = mybir.AluOpType.subtract

B, S, H, NH = 4, 2048, 1024, 16
HD = H // NH            # 64
NCORES = 8
HPC = 8                 # heads per core
C = HPC * HD            # 512 channels per core
SCALE = 1.0 / np.sqrt(HD)
N_QC = S // 128         # 16 q-chunks
N_KT = S // 128         # 16 k-tiles
N_HC = H // 128         # 8 hidden chunks
N_CB = C // 128         # 4 channel blocks
N_ST = 4                # seq tiles of 512 for Q/K proj

_CACHE = {}


def _build_nc():
    nc = bacc.Bacc(name="mha_tp2")
    x8_d = nc.dram_tensor("x8", [128, N_HC, 2, S], FP8, kind="ExternalInput")
    x16_d = nc.dram_tensor("x16", [128, N_HC, S], BF16, kind="ExternalInput")
    wq8_d = nc.dram_tensor("wq8", [128, N_HC, C], FP8, kind="ExternalInput")
    wk8_d = nc.dram_tensor("wk8", [128, N_HC, C], FP8, kind="ExternalInput")
    wvt_d = nc.dram_tensor("wvt", [128, N_HC, C], BF16, kind="ExternalInput")
    wo_d = nc.dram_tensor("wo", [128, N_CB, H], BF16, kind="ExternalInput")
    bq_d = nc.dram_tensor("bq", [128, N_CB], F32, kind="ExternalInput")
    bk_d = nc.dram_tensor("bk", [128, N_CB], F32, kind="ExternalInput")
    vb_d = nc.dram_tensor("vb", [128, HPC, HD], BF16, kind="ExternalInput")
    ut_d = nc.dram_tensor("ut", [128, 128], BF16, kind="ExternalInput")
    id_d = nc.dram_tensor("idb", [128, 128], BF16, kind="ExternalInput")
    out_d = nc.dram_tensor("out", [S, H], BF16, kind="ExternalOutput")

    with tile.TileContext(nc) as tc:
        with (
            tc.tile_pool(name="const", bufs=1) as cp,
            tc.tile_pool(name="work", bufs=2) as wp,
            tc.tile_pool(name="psA", bufs=2, space="PSUM") as psA,
            tc.tile_pool(name="psB", bufs=2, space="PSUM") as psB,
        ):
            # ---- persistent SBUF ----
            x8_s = cp.tile([128, N_HC, 2, S], FP8)
            x16_s = cp.tile([128, N_HC, S], BF16)
            wq8_s = cp.tile([128, N_HC, C], FP8)
            wk8_s = cp.tile([128, N_HC, C], FP8)
            wvt_s = cp.tile([128, N_HC, C], BF16)
            wo_s = cp.tile([128, N_CB, H], BF16)
            bq_s = cp.tile([128, N_CB], F32)
            bk_s = cp.tile([128, N_CB], F32)
            vb_s = cp.tile([128, HPC, HD], BF16)
            ut_s = cp.tile([128, 128], BF16)
            id_s = cp.tile([128, 128], BF16)
            z1_s = cp.tile([1, 128], BF16)
            z2_s = cp.tile([1, 512], BF16)
            q8_s = cp.tile([128, N_CB, S], FP8)
            k8_s = cp.tile([128, N_CB, 2, S], FP8)
            q8o_s = cp.tile([64, N_CB, S], FP8)
            k8o_s = cp.tile([64, N_CB, 2, S], FP8)
            vt_s = cp.tile([128, N_KT, HPC, HD + 1], BF16)
            ctx_s = cp.tile([128, N_CB, S], BF16)

            # DMA order tuned for ramp: K weights + first x8 chunk first so
            # the first projection units can start ~5us in.
            nc.sync.dma_start(wk8_s[:], wk8_d.ap())
            nc.vector.memset(vt_s[:, :, :, HD:HD + 1], 1.0)
            nc.vector.memset(z1_s[:], 0.0)
            nc.vector.memset(z2_s[:], 0.0)

            def dma_x8(lo, hi):
                ssl = slice(lo * 128, hi * 128)
                nc.sync.dma_start(x8_s[:, :, :, ssl], x8_d.ap()[:, :, :, ssl])

            def dma_x16(lo, hi):
                ssl = slice(lo * 128, hi * 128)
                nc.sync.dma_start(x16_s[:, :, ssl], x16_d.ap()[:, :, ssl])

            dma_x8(0, 4)
            for w_s, w_d in ((bq_s, bq_d), (bk_s, bk_d), (vb_s, vb_d),
                             (ut_s, ut_d), (id_s, id_d)):
                nc.sync.dma_start(w_s[:], w_d.ap())
            nc.sync.dma_start(wq8_s[:], wq8_d.ap())
            dma_x16(0, 4)
            nc.sync.dma_start(wvt_s[:], wvt_d.ap())
            # the rest of the loads are emitted just-in-time (as fillers on
            # the gpsimd SWDGE queue) so the shared DMA FIFO stays shallow
            # and attention-critical transfers are not stuck behind them.
            late_dmas = [
                (1, lambda: nc.sync.dma_start(wo_s[:], wo_d.ap())),
                (1, lambda: nc.sync.dma_start(
                    x8_s[:, :, :, 512:1024], x8_d.ap()[:, :, :, 512:1024])),
                (1, lambda: nc.sync.dma_start(
                    x16_s[:, :, 512:1024], x16_d.ap()[:, :, 512:1024])),
                (4, lambda: nc.sync.dma_start(
                    x8_s[:, :, :, 1024:1536], x8_d.ap()[:, :, :, 1024:1536])),
                (4, lambda: nc.sync.dma_start(
                    x16_s[:, :, 1024:1536], x16_d.ap()[:, :, 1024:1536])),
                (7, lambda: nc.sync.dma_start(
                    x8_s[:, :, :, 1536:2048], x8_d.ap()[:, :, :, 1536:2048])),
                (7, lambda: nc.sync.dma_start(
                    x16_s[:, :, 1536:2048], x16_d.ap()[:, :, 1536:2048])),
            ]

            def emit_projqk_unit(st, which, cb):
                # one (512-seq-tile, weight, chan-block) projection unit
                ssl = slice(st * 512, (st + 1) * 512)
                w_s, dst, b_s, comp = (
                    (wk8_s, k8_s, bk_s, True) if which == "k"
                    else (wq8_s, q8_s, bq_s, False))
                pp = psB.tile([128, 512], F32, tag="op", bufs=2,
                              name=f"pp{which}{st}_{cb}")
                for hc in range(N_HC):
                    nc.tensor.matmul(
                        pp[:],
                        w_s[:, hc, cb * 128:(cb + 1) * 128].unsqueeze(1)
                            .broadcast_to([128, 2, 128]),
                        x8_s[:, hc, :, ssl],
                        start=(hc == 0), stop=(hc == N_HC - 1),
                        perf_mode=DR)
                if comp:
                    nc.vector.tensor_scalar_add(
                        dst[:, cb, 0, ssl], pp[:], b_s[:, cb:cb + 1])
                    nc.vector.scalar_tensor_tensor(
                        dst[:, cb, 1, ssl], pp[:], b_s[:, cb:cb + 1],
                        dst[:, cb, 0, ssl], ADD, SUB)
                    nc.sync.dma_start(k8o_s[:, cb, :, ssl],
                                      dst[64:128, cb, :, ssl])
                else:
                    nc.vector.tensor_scalar_add(
                        dst[:, cb, ssl], pp[:], b_s[:, cb:cb + 1])
                    nc.sync.dma_start(q8o_s[:, cb, ssl],
                                      dst[64:128, cb, ssl])

            def emit_projv(kt):
                # V^T tile for k positions [kt*128, (kt+1)*128)
                pp = psA.tile([128, 512], F32, tag="sc", name=f"pv{kt}")
                for hc in range(N_HC):
                    nc.tensor.matmul(
                        pp[:], x16_s[:, hc, kt * 128:(kt + 1) * 128],
                        wvt_s[:, hc, :],
                        start=(hc == 0), stop=(hc == N_HC - 1))
                nc.vector.tensor_tensor(
                    vt_s[:, kt, :, 0:HD],
                    pp[:].rearrange("p (h d) -> p h d", h=HPC), vb_s[:], ADD)

            # ---- attention ----
            # per (qc, kc): QK for 8 heads -> one exp -> 8 PV matmuls.
            # software pipelined one block deep: PV(block i) is emitted after
            # QK(block i+1) so exp(i) overlaps PE work.  PE idle inside the
            # ACT-bound attention loop is filled from a queue of projection /
            # output-projection units.
            pend_q = []   # [(p_tile, kc, qc, accs)] pipeline, depth 2
            acc_of = {}
            fillers = []  # (needed_by_row, emit_fn)

            def emit_norm(qc):
                accs = acc_of.pop(qc)
                ctxT = wp.tile([128, HPC, HD], BF16, tag="ctxT", name=f"cT{qc}")
                for i, a in enumerate(accs):
                    av = a[:, 0:4 * 65].rearrange("p (h e) -> p h e", e=65)
                    denr = wp.tile([128, 4], F32, tag="denr", bufs=4,
                                   name=f"dn{qc}_{i}")
                    nc.vector.reciprocal(denr[:], av[:, :, HD])
                    nc.vector.tensor_tensor(
                        ctxT[:, 4 * i:4 * i + 4, :], av[:, :, 0:HD],
                        denr[:].unsqueeze(2).broadcast_to([128, 4, HD]), MUL)
                # transpose [q, c] -> [c, q] on the DMA crossbar (off PE)
                nc.sync.dma_start_transpose(
                    ctx_s[:, :, qc * 128:(qc + 1) * 128], ctxT[:])
                for oh in range(2):
                    fillers.append((qc + 4, 900.0,
                                    lambda qc=qc, oh=oh: emit_oproj(qc, oh)))

            def flush_pv():
                if not pend_q:
                    return
                p_t, kc, qc, accs = pend_q.pop(0)
                if kc == 0:
                    for a in accs:
                        nc.tensor.matmul(a[:], z1_s[:], z2_s[:],
                                         start=True, stop=True)
                for h in range(HPC):
                    a = accs[h // 4]
                    nc.tensor.matmul(
                        a[:, 65 * (h % 4):65 * (h % 4) + 65],
                        p_t[:, h, :], vt_s[:, kc, h, :],
                        start=False, stop=(kc == qc),
                        skip_group_check=True)
                if kc == qc:
                    emit_norm(qc)

            def emit_qk(qc, kc):
                if kc == 0:
                    accA = psB.tile([128, 512], F32, tag="accA", bufs=1,
                                    name=f"aA{qc}")
                    accB = psB.tile([128, 512], F32, tag="accB", bufs=1,
                                    name=f"aB{qc}")
                    acc_of[qc] = (accA, accB)
                accs = acc_of[qc]
                sc_t = psA.tile([128, HPC, 128], F32, tag="sc",
                                name=f"s{qc}_{kc}")
                qsl = slice(qc * 128, (qc + 1) * 128)
                ksl = slice(kc * 128, (kc + 1) * 128)
                for h in range(HPC):
                    cb = h // 2
                    if h % 2 == 0:
                        k_ap = k8_s[0:64, cb, :, ksl]
                        q_ap = q8_s[0:64, cb, qsl]
                    else:
                        k_ap = k8o_s[:, cb, :, ksl]
                        q_ap = q8o_s[:, cb, qsl]
                    nc.tensor.matmul(
                        sc_t[:, h, :], k_ap,
                        q_ap.unsqueeze(1).broadcast_to([64, 2, 128]),
                        start=True, stop=True, perf_mode=DR)
                p_t = wp.tile([128, HPC, 128], BF16, tag="p", bufs=14,
                              name=f"p{qc}_{kc}")
                nc.scalar.activation(p_t[:], sc_t[:], AF.Exp, scale=float(SCALE))
                if kc == qc:
                    nc.vector.tensor_tensor(
                        p_t[:], p_t[:],
                        ut_s[:].unsqueeze(1).broadcast_to([128, HPC, 128]), MUL)
                pend_q.append((p_t, kc, qc, accs))

            ob_of = {}

            def emit_oproj(sc, oh):
                ssl = slice(sc * 128, (sc + 1) * 128)
                if sc not in ob_of:
                    ob_of[sc] = wp.tile([128, H], BF16, tag="ob",
                                        name=f"ob{sc}")
                ob = ob_of[sc]
                osl = slice(oh * 512, (oh + 1) * 512)
                op = psB.tile([128, 512], F32, tag="op", name=f"o{sc}_{oh}")
                for cb in range(N_CB):
                    nc.tensor.matmul(op[:], ctx_s[:, cb, ssl],
                                     wo_s[:, cb, osl],
                                     start=(cb == 0), stop=(cb == N_CB - 1))
                nc.vector.tensor_copy(ob[:, osl], op[:])
                if oh == 1:
                    nc.sync.dma_start(out_d.ap()[ssl, :], ob[:])
                    del ob_of[sc]

            fill_ns = [0.0]     # filler PE-time emitted so far
            blocks = [0]        # attention blocks emitted so far
            SLACK_NS = 620.0    # PE slack per ACT-bound block

            def pop_fillers(row, opportunistic):
                # emit everything required by this row; then, if opportunistic,
                # fillers up to the cumulative PE-slack budget
                i = 0
                while i < len(fillers):
                    nb, cost, fn = fillers[i]
                    if nb <= row:
                        fillers.pop(i)
                        fill_ns[0] += cost
                        fn()
                    else:
                        i += 1
                slack = 1100.0 if row < 8 else SLACK_NS
                while (opportunistic and fillers
                       and fill_ns[0] < blocks[0] * slack):
                    nb, cost, fn = fillers.pop(0)
                    fill_ns[0] += cost
                    fn()

            # ---- emission schedule ----
            for wu in range(18):
                wt = psB.tile([128, 512], F32, tag="op", name=f"wu{wu}")
                nc.tensor.matmul(wt[:], z1_s[:], z2_s[:], start=True,
                                 stop=True)
            for nb, fn in late_dmas:
                fillers.append((nb, 0.0, fn))
            for st in range(N_ST):
                for cb in range(N_CB):
                    fillers.append((4 * st, 900.0, lambda st=st, cb=cb:
                                    emit_projqk_unit(st, "k", cb)))
                    fillers.append((4 * st, 900.0, lambda st=st, cb=cb:
                                    emit_projqk_unit(st, "q", cb)))
            for kt in range(N_KT):
                fillers.append((kt, 1750.0, lambda kt=kt: emit_projv(kt)))
            fillers.sort(key=lambda f: f[0])

            for qc in range(N_QC):
                pop_fillers(qc, False)   # force units this row needs
                depth = 9 if qc < 14 else 2
                for kc in range(qc + 1):
                    emit_qk(qc, kc)
                    blocks[0] += 1
                    while len(pend_q) > depth:
                        flush_pv()
                    pop_fillers(qc, True)
            while pend_q:
                flush_pv()
            pop_fillers(999, False)
            while fillers:
                fillers.pop(0)[2]()

    nc.compile()
    return nc


def _get_nc():
    if "nc" not in _CACHE:
        _CACHE["nc"] = _build_nc()
    return _CACHE["nc"]


def _e4(a):
    return a.astype(ml_dtypes.float8_e4m3)


def make_in_maps(x, Wq, bq, Wk, bk, Wv, bv, Wo):
    x = np.asarray(x, np.float32)
    in_maps = []
    ut = np.triu(np.ones((128, 128), np.float32)).astype(ml_dtypes.bfloat16)
    idb = np.eye(128, dtype=ml_dtypes.bfloat16)
    for core in range(NCORES):
        b, g = core // 2, core % 2
        csl = slice(g * C, (g + 1) * C)
        xT = np.ascontiguousarray(x[b].T)                       # [H, S]
        x_hi = _e4(xT)
        x_lo = _e4(xT - x_hi.astype(np.float32))
        x8 = np.stack([x_hi.reshape(N_HC, 128, S),
                       x_lo.reshape(N_HC, 128, S)], axis=2)     # [hc,128,2,S]
        x8 = np.ascontiguousarray(x8.transpose(1, 0, 2, 3))     # [128,hc,2,S]
        x16 = np.ascontiguousarray(
            xT.astype(ml_dtypes.bfloat16).reshape(N_HC, 128, S)
            .transpose(1, 0, 2))
        def wlayout(W):
            W8 = _e4(np.asarray(W, np.float32)[csl, :].T)       # [H, C]
            return np.ascontiguousarray(
                W8.reshape(N_HC, 128, C).transpose(1, 0, 2))
        wvt = (np.asarray(Wv, np.float32)[csl, :].T
               .astype(ml_dtypes.bfloat16).reshape(N_HC, 128, C)
               .transpose(1, 0, 2))
        wo = (np.asarray(Wo, np.float32)[:, csl].T
              .astype(ml_dtypes.bfloat16).reshape(N_CB, 128, H)
              .transpose(1, 0, 2))
        vb = np.broadcast_to(
            np.asarray(bv, np.float32)[csl].reshape(HPC, HD), (128, HPC, HD))
        in_maps.append({
            "x8": x8,
            "x16": np.ascontiguousarray(x16),
            "wq8": wlayout(Wq),
            "wk8": wlayout(Wk),
            "wvt": np.ascontiguousarray(wvt),
            "wo": np.ascontiguousarray(wo),
            "bq": np.ascontiguousarray(
                np.asarray(bq, np.float32)[csl].reshape(N_CB, 128).T),
            "bk": np.ascontiguousarray(
                np.asarray(bk, np.float32)[csl].reshape(N_CB, 128).T),
            "vb": np.ascontiguousarray(vb.astype(ml_dtypes.bfloat16)),
            "ut": ut,
            "idb": idb,
        })
    return in_maps


def unshard(partials, bo):
    """partials: list of 8 [S, H] bf16 arrays -> full [B, S, H] f32."""
    out = np.zeros((B, S, H), np.float32)
    for core, p in enumerate(partials):
        out[core // 2] += np.asarray(p, dtype=np.float32)
    return out + np.asarray(bo, np.float32)[None, None, :]


def kernel(x, mask, Wq, bq, Wk, bk, Wv, bv, Wo, bo):
    nc = _get_nc()
    in_maps = make_in_maps(x, Wq, bq, Wk, bk, Wv, bv, Wo)
    res = run_bass_kernel_spmd(nc, in_maps, core_ids=list(range(NCORES)))
    return unshard([r["out"] for r in res.results], bo).astype(np.float32)


if __name__ == "__main__":
    nc = _get_nc()
    from concourse.timeline_sim import TimelineSim
    print("sim ns:", TimelineSim(nc, trace=False).simulate())
